# revision 9
# baseline (speedup 1.0000x reference)
"""Trainium2 Bass kernel for a BailingMoE sparse-MoE block (T=512, H=2048,
E=16 experts top-4 renormalized, expert FFN I=1408, shared expert IS=2816).

Strategy (8 NeuronCores, SPMD, no collectives):
  * Expert-parallel: core c owns experts {2c, 2c+1}; shared expert is
    TP-sharded over its intermediate dim (padded 2816->3072, 384 ch/core).
  * Router is computed on-device (replicated on every core).
  * Sparse token dispatch on-device via matmul with one-hot dispatch
    matrices built from a cumsum (triangular matmul) of the top-4 mask.
  * Weights are streamed from HBM as bf16 (host downcast); activations bf16,
    all matmul accumulation in fp32 PSUM. Router math is fp32.
  * Each core writes a full [T, H] fp32 partial; the host sums the 8 partials.

The uniform per-expert capacity C is chosen on the host from the actual
routing counts (rounded up); the same compiled graph runs on all cores.
"""

import math

import numpy as np
import ml_dtypes

import concourse.bass as bass
import concourse.mybir as mybir
import concourse.tile as tile
from concourse import bacc
from concourse.bass import ts, ds
from concourse.bass_utils import run_bass_kernel_spmd
from concourse.masks import make_identity

F32 = mybir.dt.float32
BF16 = mybir.dt.bfloat16
BF = ml_dtypes.bfloat16

T, H, E, K, I, IS = 512, 2048, 16, 4, 1408, 2816
NCORES = 8
EPC = E // NCORES            # experts per core
ISP = 3072                   # padded shared intermediate (divisible by 8*128)
ISC = ISP // NCORES          # shared channels per core (384 = 3 tiles)
TT = T // 128                # 4 token tiles
HT = H // 128                # 16 hidden chunks
HK = H // 512                # 4 hidden 512-chunks
IT = I // 128                # 11 expert-intermediate tiles
JSH = ISC // 128             # 3 shared-intermediate tiles per core

AX = mybir.AxisListType
ALU = mybir.AluOpType
ACTF = mybir.ActivationFunctionType


def build_nc(C: int):
    """Build the SPMD single-core graph with uniform expert capacity C."""
    assert C % 64 == 0 and 64 <= C <= 512
    CT = math.ceil(C / 128)
    csz = [min(128, C - ct * 128) for ct in range(CT)]

    nc = bacc.Bacc("TRN2", target_bir_lowering=False, debug=False)

    xt_f32_d = nc.dram_tensor("xt_f32", [HT, 128, T], F32, kind="ExternalInput")
    gw_t_d = nc.dram_tensor("gw_t", [HT, 128, E], F32, kind="ExternalInput")
    x_bf_d = nc.dram_tensor("x_bf", [TT, 128, H], BF16, kind="ExternalInput")
    xt_bf_d = nc.dram_tensor("xt_bf", [HT, 128, T], BF16, kind="ExternalInput")
    tri_d = nc.dram_tensor("tri", [TT, 128, T], BF16, kind="ExternalInput")
    iota_d = nc.dram_tensor("iota_row", [128, T], F32, kind="ExternalInput")
    wgu_d = nc.dram_tensor("wgu", [EPC, 2 * IT, 128, H], BF16, kind="ExternalInput")
    wd_d = nc.dram_tensor("wd", [EPC, IT, 128, H], BF16, kind="ExternalInput")
    swgu_d = nc.dram_tensor("swgu", [2 * JSH, 128, H], BF16, kind="ExternalInput")
    swd_d = nc.dram_tensor("swd", [JSH, 128, H], BF16, kind="ExternalInput")
    out_d = nc.dram_tensor("out", [T, H], F32, kind="ExternalOutput")

    with tile.TileContext(nc) as tc:
        with (
            tc.tile_pool(name="consts", bufs=1) as consts,
            tc.tile_pool(name="persist", bufs=1) as persist,
            tc.tile_pool(name="wpool", bufs=3) as wpool,
            tc.tile_pool(name="hpool", bufs=2) as hpool,
            tc.tile_pool(name="ypool", bufs=1) as ypool,
            tc.tile_pool(name="rsb", bufs=2) as rsb,
        ):
            ident_bf = consts.tile([128, 128], BF16)
            make_identity(nc, ident_bf)
            ident_f = consts.tile([128, 128], F32)
            make_identity(nc, ident_f)

            # resident inputs
            x_sb = persist.tile([128, TT, H], BF16)
            nc.sync.dma_start(x_sb, x_bf_d[:].rearrange("a p f -> p a f"))
            xt_sb = persist.tile([128, HT, T], BF16)
            nc.sync.dma_start(xt_sb, xt_bf_d[:].rearrange("a p f -> p a f"))
            tri_sb = persist.tile([128, TT, T], BF16)
            nc.sync.dma_start(tri_sb, tri_d[:].rearrange("a p f -> p a f"))
            iota_sb = persist.tile([128, T], F32)
            nc.sync.dma_start(iota_sb, iota_d[:])
            gw_sb = persist.tile([128, HT, E], F32)
            nc.sync.dma_start(gw_sb, gw_t_d[:].rearrange("a p f -> p a f"))
            swd_sb = persist.tile([128, JSH, H], BF16)
            nc.sync.dma_start(swd_sb, swd_d[:].rearrange("a p f -> p a f"))

            # router state
            cw = persist.tile([128, TT, E], F32)       # renormalized top-4 weights
            mask_f = persist.tile([128, TT, E], F32)   # {0,1} top-4 mask
            mask_bf = persist.tile([128, TT, E], BF16)
            pos = persist.tile([128, TT, E], F32)      # exclusive cumsum slots

            # ---------------- router ----------------
            with tc.tile_pool(name="pr", space="PSUM", bufs=2) as pr:
                lg_ps = pr.tile([16, T], F32, tag="lgT")
                for hc in range(HT):
                    xtf = rsb.tile([128, T], F32, tag="xtf")
                    nc.sync.dma_start(xtf, xt_f32_d[hc])
                    nc.tensor.matmul(
                        lg_ps, gw_sb[:, hc], xtf,
                        start=(hc == 0), stop=(hc == HT - 1),
                    )
                lgT_sb = rsb.tile([16, T], F32, tag="lgTs")
                nc.vector.tensor_copy(lgT_sb, lg_ps)

                for tt in range(TT):
                    lg2 = pr.tile([128, E], F32, tag="lg")
                    nc.tensor.transpose(
                        lg2, lgT_sb[:, ts(tt, 128)], ident_f[:16, :16]
                    )
                    rm = rsb.tile([128, 1], F32, tag="rm")
                    nc.vector.tensor_reduce(
                        rm, lg2, axis=AX.X, op=ALU.max, negate=True
                    )
                    ex = rsb.tile([128, E], F32, tag="ex")
                    nc.scalar.activation(ex, lg2, ACTF.Exp, bias=rm, scale=1.0)
                    m8 = rsb.tile([128, 8], F32, tag="m8")
                    nc.vector.max(m8, ex)
                    nc.vector.tensor_scalar(
                        mask_f[:, tt], ex, m8[:, 3:4], None, op0=ALU.is_ge
                    )
                    cwr = rsb.tile([128, E], F32, tag="cwr")
                    nc.vector.tensor_mul(cwr, ex, mask_f[:, tt])
                    s4 = rsb.tile([128, 1], F32, tag="s4")
                    nc.vector.tensor_reduce(s4, cwr, axis=AX.X, op=ALU.add)
                    rs4 = rsb.tile([128, 1], F32, tag="rs4")
                    nc.vector.reciprocal(rs4, s4)
                    nc.vector.tensor_scalar_mul(cw[:, tt], cwr, rs4)
                    nc.vector.tensor_copy(mask_bf[:, tt], mask_f[:, tt])

                # exclusive cumsum over tokens (per expert) via triangular matmul
                for tt in range(TT):
                    pos_ps = pr.tile([128, E], F32, tag="pos")
                    for tc_ in range(tt + 1):
                        nc.tensor.matmul(
                            pos_ps,
                            tri_sb[:, tc_, ts(tt, 128)],
                            mask_bf[:, tc_],
                            start=(tc_ == 0), stop=(tc_ == tt),
                        )
                    nc.vector.tensor_copy(pos[:, tt], pos_ps)

            # ------------- dispatch matrices + gathered tokens -------------
            # D[t, c] (bf16) per expert+t-tile; Dpw[c, t] weighted transpose
            Dpw = persist.tile([128, EPC, CT, T], BF16)
            xd = persist.tile([128, EPC, HT, C], BF16)
            with (
                tc.tile_pool(name="pd", space="PSUM", bufs=2) as pd,
                tc.tile_pool(name="dsb", bufs=5) as dsb,
            ):
                for e in range(EPC):
                    Dts = []
                    for tt in range(TT):
                        Dt = dsb.tile([128, C], BF16, tag=f"D{tt}")
                        # (iota == pos) * mask
                        nc.vector.tensor_scalar(
                            Dt, iota_sb[:, :C],
                            pos[:, tt, e:e + 1], mask_f[:, tt, e:e + 1],
                            op0=ALU.is_equal, op1=ALU.mult,
                        )
                        Dts.append(Dt)
                        Dwt = dsb.tile([128, C], BF16, tag="Dw")
                        nc.vector.tensor_scalar_mul(
                            Dwt, Dt, cw[:, tt, e:e + 1]
                        )
                        for ct in range(CT):
                            tp = pd.tile([csz[ct], 128], BF16, tag="tp")
                            nc.tensor.transpose(
                                tp, Dwt[:, ds(ct * 128, csz[ct])], ident_bf
                            )
                            nc.vector.tensor_copy(
                                Dpw[:csz[ct], e, ct, ts(tt, 128)], tp
                            )
                    for hc in range(HT):
                        xd_ps = pd.tile([128, C], F32, tag="xd")
                        for tc_ in range(TT):
                            nc.tensor.matmul(
                                xd_ps,
                                x_sb[:, tc_, ts(hc, 128)],
                                Dts[tc_],
                                start=(tc_ == 0), stop=(tc_ == TT - 1),
                            )
                        nc.vector.tensor_copy(xd[:, e, hc], xd_ps)

            # ------------- experts: gate_up -> silu*up -> down -------------
            y_tiles = {}
            for e in range(EPC):
                h_sb = hpool.tile([128, IT, C], BF16, tag="h")
                with tc.tile_pool(name=f"pgu{e}", space="PSUM", bufs=4) as pgu:
                    for j in range(IT):
                        wg = wpool.tile([128, 2, H], BF16, tag="wgu")
                        nc.sync.dma_start(
                            wg, wgu_d[e, 2 * j:2 * j + 2].rearrange("a p f -> p a f")
                        )
                        ps_g = pgu.tile([128, C], F32, tag="gu")
                        ps_u = pgu.tile([128, C], F32, tag="gu")
                        for hc in range(HT):
                            nc.tensor.matmul(
                                ps_g, wg[:, 0, ts(hc, 128)], xd[:, e, hc],
                                start=(hc == 0), stop=(hc == HT - 1),
                            )
                        for hc in range(HT):
                            nc.tensor.matmul(
                                ps_u, wg[:, 1, ts(hc, 128)], xd[:, e, hc],
                                start=(hc == 0), stop=(hc == HT - 1),
                            )
                        sg = rsb.tile([128, C], BF16, tag="sg")
                        nc.scalar.activation(sg, ps_g, ACTF.Sigmoid)
                        sg2 = rsb.tile([128, C], BF16, tag="sg2")
                        nc.vector.tensor_mul(sg2, sg, ps_g)
                        nc.vector.tensor_mul(h_sb[:, j], sg2, ps_u)

                for ct in range(CT):
                    y_tiles[(e, ct)] = ypool.tile(
                        [csz[ct], H], BF16, tag=f"y{e}{ct}", name=f"y{e}{ct}"
                    )
                with tc.tile_pool(name=f"py{e}", space="PSUM", bufs=1) as py:
                    ps_y = {
                        (ct, hk): py.tile(
                            [csz[ct], 512], F32,
                            tag=f"py{ct}{hk}", name=f"psy{ct}{hk}",
                        )
                        for ct in range(CT) for hk in range(HK)
                    }
                    for ic in range(IT):
                        wdt = wpool.tile([128, H], BF16, tag="wd")
                        nc.sync.dma_start(wdt, wd_d[e, ic])
                        for ct in range(CT):
                            for hk in range(HK):
                                nc.tensor.matmul(
                                    ps_y[(ct, hk)],
                                    h_sb[:, ic, ds(ct * 128, csz[ct])],
                                    wdt[:, ts(hk, 512)],
                                    start=(ic == 0), stop=(ic == IT - 1),
                                )
                    for ct in range(CT):
                        for hk in range(HK):
                            nc.vector.tensor_copy(
                                y_tiles[(e, ct)][:, ts(hk, 512)], ps_y[(ct, hk)]
                            )

            # ------------- shared expert (TP shard) -------------
            hsh = persist.tile([128, JSH, T], BF16)
            with tc.tile_pool(name="pgsh", space="PSUM", bufs=4) as pgsh:
                for j in range(JSH):
                    wg = wpool.tile([128, 2, H], BF16, tag="wgu")
                    nc.sync.dma_start(
                        wg, swgu_d[2 * j:2 * j + 2].rearrange("a p f -> p a f")
                    )
                    ps_g = pgsh.tile([128, T], F32, tag="gush")
                    ps_u = pgsh.tile([128, T], F32, tag="gush")
                    for hc in range(HT):
                        nc.tensor.matmul(
                            ps_g, wg[:, 0, ts(hc, 128)], xt_sb[:, hc],
                            start=(hc == 0), stop=(hc == HT - 1),
                        )
                    for hc in range(HT):
                        nc.tensor.matmul(
                            ps_u, wg[:, 1, ts(hc, 128)], xt_sb[:, hc],
                            start=(hc == 0), stop=(hc == HT - 1),
                        )
                    sg = rsb.tile([128, T], BF16, tag="sgsh")
                    nc.scalar.activation(sg, ps_g, ACTF.Sigmoid)
                    sg2 = rsb.tile([128, T], BF16, tag="sgsh2")
                    nc.vector.tensor_mul(sg2, sg, ps_g)
                    nc.vector.tensor_mul(hsh[:, j], sg2, ps_u)

            # ------------- combine: routed (weighted) + shared -------------
            with (
                tc.tile_pool(name="po", space="PSUM", bufs=4) as po,
                tc.tile_pool(name="osb", bufs=4) as osb,
            ):
                chain = [("sh", j) for j in range(JSH)] + [
                    (e, ct) for e in range(EPC) for ct in range(CT)
                ]
                for tt in range(TT):
                    for hk in range(HK):
                        ps_o = po.tile([128, 512], F32, tag="o")
                        for n, (a, b) in enumerate(chain):
                            st, sp = (n == 0), (n == len(chain) - 1)
                            if a == "sh":
                                nc.tensor.matmul(
                                    ps_o,
                                    hsh[:, b, ts(tt, 128)],
                                    swd_sb[:, b, ts(hk, 512)],
                                    start=st, stop=sp,
                                )
                            else:
                                nc.tensor.matmul(
                                    ps_o,
                                    Dpw[:csz[b], a, b, ts(tt, 128)],
                                    y_tiles[(a, b)][:, ts(hk, 512)],
                                    start=st, stop=sp,
                                )
                        o_sb = osb.tile([128, 512], F32, tag="o")
                        nc.vector.tensor_copy(o_sb, ps_o)
                        nc.sync.dma_start(
                            out_d[ts(tt, 128), ts(hk, 512)], o_sb
                        )
    nc.compile()
    return nc


def _lhsT_tiles(Wt: np.ndarray, col0: int) -> np.ndarray:
    """Wt: [H, cols] fp32/bf16. Returns [128, H] where element (p, k*128+c) =
    Wt[k*128+p, col0+c] — i.e. the lhsT chunk layout for 16 h-chunks."""
    blk = Wt[:, col0:col0 + 128].reshape(HT, 128, 128)
    return np.ascontiguousarray(blk.transpose(1, 0, 2)).reshape(128, H)


def _route_capacity(x: np.ndarray, gate_w: np.ndarray) -> int:
    logits = x.astype(np.float64) @ gate_w.T.astype(np.float64)
    part = np.partition(logits, E - K - 1, axis=-1)
    thr = part[:, E - K - 1]  # (K+1)-th largest == just below the top-K
    counts = (logits > thr[:, None]).sum(0)
    c = int(counts.max()) + 8  # safety margin for fp32-vs-fp64 boundary flips
    return min(512, max(64, ((c + 63) // 64) * 64))


_BUILD_CACHE = {}


def prepare(
    hidden_states, gate_w, w_gate_up, w_down, shared_gate_up, shared_down
):
    """Host-side sharding/layout prep. Returns (C, in_maps)."""
    x = np.ascontiguousarray(np.asarray(hidden_states, dtype=np.float32))
    gate_w = np.asarray(gate_w, dtype=np.float32)
    w_gate_up = np.asarray(w_gate_up, dtype=np.float32)
    w_down = np.asarray(w_down, dtype=np.float32)
    shared_gate_up = np.asarray(shared_gate_up, dtype=np.float32)
    shared_down = np.asarray(shared_down, dtype=np.float32)

    C = _route_capacity(x, gate_w)

    # --- common (replicated) host-side layouts ---
    xt = np.ascontiguousarray(x.T)                        # [H, T]
    xt_f32 = xt.reshape(HT, 128, T)
    xt_bf = xt_f32.astype(BF)
    x_bf = x.reshape(TT, 128, H).astype(BF)
    tri = np.triu(np.ones((T, T), np.float32), 1).reshape(TT, 128, T).astype(BF)
    iota_row = np.broadcast_to(
        np.arange(T, dtype=np.float32), (128, T)
    ).copy()

    # shared expert: pad IS -> ISP and shard
    sg_T = np.zeros((H, ISP), np.float32)
    sg_T[:, :IS] = shared_gate_up[:IS].T
    su_T = np.zeros((H, ISP), np.float32)
    su_T[:, :IS] = shared_gate_up[IS:].T
    sd_T = np.zeros((ISP, H), np.float32)
    sd_T[:IS] = shared_down.T

    in_maps = []
    for c in range(NCORES):
        e0 = EPC * c
        # The device graph reads router columns 0..EPC-1 as "this core's
        # experts": permute gate_w rows so global experts (2c, 2c+1) land
        # in columns 0,1 (softmax/top-k/cumsum are column-order invariant).
        perm = [e0 + el for el in range(EPC)] + [
            e for e in range(E) if not (e0 <= e < e0 + EPC)
        ]
        gw_t = np.ascontiguousarray(gate_w[perm].T).reshape(HT, 128, E)
        wgu = np.empty((EPC, 2 * IT, 128, H), BF)
        wd = np.empty((EPC, IT, 128, H), BF)
        for el in range(EPC):
            Wt = w_gate_up[e0 + el].T.astype(np.float32)  # [H, 2I]
            for j in range(IT):
                wgu[el, 2 * j] = _lhsT_tiles(Wt, j * 128).astype(BF)
                wgu[el, 2 * j + 1] = _lhsT_tiles(Wt, I + j * 128).astype(BF)
            WdT = w_down[e0 + el].T  # [I, H]
            for ic in range(IT):
                wd[el, ic] = WdT[ic * 128:(ic + 1) * 128].astype(BF)
        base = c * ISC
        swgu = np.empty((2 * JSH, 128, H), BF)
        for j in range(JSH):
            swgu[2 * j] = _lhsT_tiles(sg_T, base + j * 128).astype(BF)
            swgu[2 * j + 1] = _lhsT_tiles(su_T, base + j * 128).astype(BF)
        swd = sd_T[base:base + ISC].reshape(JSH, 128, H).astype(BF)

        in_maps.append({
            "xt_f32": xt_f32, "gw_t": gw_t, "x_bf": x_bf, "xt_bf": xt_bf,
            "tri": tri, "iota_row": iota_row,
            "wgu": wgu, "wd": wd, "swgu": swgu, "swd": swd,
        })
    return C, in_maps


def kernel(**inputs):
    C, in_maps = prepare(**inputs)
    if C not in _BUILD_CACHE:
        _BUILD_CACHE[C] = build_nc(C)
    nc = _BUILD_CACHE[C]

    res = run_bass_kernel_spmd(nc, in_maps, core_ids=list(range(NCORES)))
    out = np.zeros((T, H), np.float32)
    for r in res.results:
        out += r["out"]
    return out


# revision 12
# speedup vs baseline: 1.0667x; 1.0667x over previous
"""Trainium2 Bass kernel for a BailingMoE sparse-MoE block (T=512, H=2048,
E=16 experts top-4 renormalized, expert FFN I=1408, shared expert IS=2816).

Strategy (8 NeuronCores, SPMD, no collectives):
  * Expert-parallel: core c owns experts {2c, 2c+1}; shared expert is
    TP-sharded over its intermediate dim (padded 2816->3072, 384 ch/core).
  * Router is computed on-device (replicated on every core); each core's
    gate_w input is column-permuted so its own experts are columns 0..EPC-1.
  * Sparse token dispatch on-device via matmul with one-hot dispatch
    matrices built from a cumsum (triangular matmul) of the top-4 mask.
  * Weights are streamed from HBM as bf16 (host downcast); activations bf16,
    all matmul accumulation in fp32 PSUM. Router math is fp32.
  * Each core writes a full [T, H] fp32 partial; the host sums the 8 partials.

The uniform per-expert capacity C is chosen on the host from the actual
routing counts (rounded up); the same compiled graph runs on all cores.
"""

import math

import numpy as np
import ml_dtypes

import concourse.bass as bass
import concourse.mybir as mybir
import concourse.tile as tile
from concourse import bacc
from concourse.bass import ts, ds
from concourse.bass_utils import run_bass_kernel_spmd
from concourse.masks import make_identity

F32 = mybir.dt.float32
BF16 = mybir.dt.bfloat16
BF = ml_dtypes.bfloat16

T, H, E, K, I, IS = 512, 2048, 16, 4, 1408, 2816
NCORES = 8
EPC = E // NCORES            # experts per core
ISP = 3072                   # padded shared intermediate (divisible by 8*128)
ISC = ISP // NCORES          # shared channels per core (384 = 3 tiles)
TT = T // 128                # 4 token tiles
HT = H // 128                # 16 hidden chunks
HK = H // 512                # 4 hidden 512-chunks
IT = I // 128                # 11 expert-intermediate tiles
JSH = ISC // 128             # 3 shared-intermediate tiles per core

AX = mybir.AxisListType
ALU = mybir.AluOpType
ACTF = mybir.ActivationFunctionType


def build_nc(C: int):
    """Build the SPMD single-core graph with uniform expert capacity C."""
    assert C % 32 == 0 and 64 <= C <= 512
    CT = math.ceil(C / 128)
    csz = [min(128, C - ct * 128) for ct in range(CT)]

    nc = bacc.Bacc("TRN2", target_bir_lowering=False, debug=False)

    xt_f32_d = nc.dram_tensor("xt_f32", [HT, 128, T], F32, kind="ExternalInput")
    gw_t_d = nc.dram_tensor("gw_t", [HT, 128, E], F32, kind="ExternalInput")
    x_bf_d = nc.dram_tensor("x_bf", [TT, 128, H], BF16, kind="ExternalInput")
    xt_bf_d = nc.dram_tensor("xt_bf", [HT, 128, T], BF16, kind="ExternalInput")
    tri_d = nc.dram_tensor("tri", [TT, 128, T], BF16, kind="ExternalInput")
    iota_d = nc.dram_tensor("iota_row", [128, T], F32, kind="ExternalInput")
    wgu_d = nc.dram_tensor("wgu", [EPC, 2 * IT, 128, H], BF16, kind="ExternalInput")
    wd_d = nc.dram_tensor("wd", [EPC, IT, 128, H], BF16, kind="ExternalInput")
    swgu_d = nc.dram_tensor("swgu", [2 * JSH, 128, H], BF16, kind="ExternalInput")
    swd_d = nc.dram_tensor("swd", [JSH, 128, H], BF16, kind="ExternalInput")
    out_d = nc.dram_tensor("out", [T, H], F32, kind="ExternalOutput")

    with tile.TileContext(nc) as tc:
        with (
            tc.tile_pool(name="consts", bufs=1) as consts,
            tc.tile_pool(name="persist", bufs=1) as persist,
            tc.tile_pool(name="wpool", bufs=4) as wpool,
            tc.tile_pool(name="hpool", bufs=2) as hpool,
            tc.tile_pool(name="ypool", bufs=1) as ypool,
            tc.tile_pool(name="rsb", bufs=2) as rsb,
        ):
            ident_bf = consts.tile([128, 128], BF16)
            make_identity(nc, ident_bf)
            ident_f = consts.tile([128, 128], F32)
            make_identity(nc, ident_f)

            # resident inputs (xt_bf first: the shared expert consumes it and
            # is the first PE work while the router's fp32 stream lands)
            xt_sb = persist.tile([128, HT, T], BF16)
            nc.sync.dma_start(xt_sb, xt_bf_d[:].rearrange("a p f -> p a f"))
            x_sb = persist.tile([128, TT, H], BF16)
            nc.sync.dma_start(x_sb, x_bf_d[:].rearrange("a p f -> p a f"))
            tri_sb = persist.tile([128, TT, T], BF16)
            nc.sync.dma_start(tri_sb, tri_d[:].rearrange("a p f -> p a f"))
            iota_sb = persist.tile([128, T], F32)
            nc.sync.dma_start(iota_sb, iota_d[:])
            gw_sb = persist.tile([128, HT, E], F32)
            nc.sync.dma_start(gw_sb, gw_t_d[:].rearrange("a p f -> p a f"))
            swd_sb = persist.tile([128, JSH, H], BF16)
            nc.sync.dma_start(swd_sb, swd_d[:].rearrange("a p f -> p a f"))

            # ------------- shared expert gate_up (no router dependency) ----
            hsh = persist.tile([128, JSH, T], BF16)
            with tc.tile_pool(name="pgsh", space="PSUM", bufs=4) as pgsh:
                for j in range(JSH):
                    wg = wpool.tile([128, 2, H], BF16, tag="wgu", name="wgsh")
                    nc.sync.dma_start(
                        wg, swgu_d[2 * j:2 * j + 2].rearrange("a p f -> p a f")
                    )
                    ps_g = pgsh.tile([128, T], F32, tag="gush")
                    ps_u = pgsh.tile([128, T], F32, tag="gush")
                    for hc in range(HT):
                        nc.tensor.matmul(
                            ps_g, wg[:, 0, ts(hc, 128)], xt_sb[:, hc],
                            start=(hc == 0), stop=(hc == HT - 1),
                        )
                    for hc in range(HT):
                        nc.tensor.matmul(
                            ps_u, wg[:, 1, ts(hc, 128)], xt_sb[:, hc],
                            start=(hc == 0), stop=(hc == HT - 1),
                        )
                    sg = rsb.tile([128, T], BF16, tag="sgsh")
                    nc.scalar.activation(sg, ps_g, ACTF.Sigmoid)
                    sg2 = rsb.tile([128, T], BF16, tag="sgsh2")
                    nc.vector.tensor_mul(sg2, sg, ps_g)
                    nc.vector.tensor_mul(hsh[:, j], sg2, ps_u)

            # router state
            cw = persist.tile([128, TT, E], F32)       # renormalized top-4 weights
            mask_f = persist.tile([128, TT, E], F32)   # {0,1} top-4 mask
            mask_bf = persist.tile([128, TT, E], BF16)
            pos = persist.tile([128, TT, E], F32)      # exclusive cumsum slots

            # ---------------- router ----------------
            with tc.tile_pool(name="pr", space="PSUM", bufs=2) as pr:
                lg_ps = pr.tile([16, T], F32, tag="lgT")
                for q in range(4):  # four 1 MiB chunks of x^T (fp32)
                    xtf = rsb.tile([128, 4, T], F32, tag="xtf")
                    nc.sync.dma_start(
                        xtf, xt_f32_d[4 * q:4 * q + 4].rearrange("a p f -> p a f")
                    )
                    for k in range(4):
                        hc = 4 * q + k
                        nc.tensor.matmul(
                            lg_ps, gw_sb[:, hc], xtf[:, k],
                            start=(hc == 0), stop=(hc == HT - 1),
                        )
                lgT_sb = rsb.tile([16, T], F32, tag="lgTs")
                nc.vector.tensor_copy(lgT_sb, lg_ps)

                for tt in range(TT):
                    lg2 = pr.tile([128, E], F32, tag="lg")
                    nc.tensor.transpose(
                        lg2, lgT_sb[:, ts(tt, 128)], ident_f[:16, :16]
                    )
                    rm = rsb.tile([128, 1], F32, tag="rm")
                    nc.vector.tensor_reduce(
                        rm, lg2, axis=AX.X, op=ALU.max, negate=True
                    )
                    ex = rsb.tile([128, E], F32, tag="ex")
                    nc.scalar.activation(ex, lg2, ACTF.Exp, bias=rm, scale=1.0)
                    m8 = rsb.tile([128, 8], F32, tag="m8")
                    nc.vector.max(m8, ex)
                    nc.vector.tensor_scalar(
                        mask_f[:, tt], ex, m8[:, 3:4], None, op0=ALU.is_ge
                    )
                    cwr = rsb.tile([128, E], F32, tag="cwr")
                    nc.vector.tensor_mul(cwr, ex, mask_f[:, tt])
                    s4 = rsb.tile([128, 1], F32, tag="s4")
                    nc.vector.tensor_reduce(s4, cwr, axis=AX.X, op=ALU.add)
                    rs4 = rsb.tile([128, 1], F32, tag="rs4")
                    nc.vector.reciprocal(rs4, s4)
                    nc.vector.tensor_scalar_mul(cw[:, tt], cwr, rs4)
                    nc.vector.tensor_copy(mask_bf[:, tt], mask_f[:, tt])

                # exclusive cumsum over tokens (per expert) via triangular matmul
                for tt in range(TT):
                    pos_ps = pr.tile([128, E], F32, tag="pos")
                    for tc_ in range(tt + 1):
                        nc.tensor.matmul(
                            pos_ps,
                            tri_sb[:, tc_, ts(tt, 128)],
                            mask_bf[:, tc_],
                            start=(tc_ == 0), stop=(tc_ == tt),
                        )
                    nc.vector.tensor_copy(pos[:, tt], pos_ps)

            # ------------- dispatch matrices + gathered tokens -------------
            # D[t, c] (bf16) per expert+t-tile; Dpw[c, t] weighted transpose
            Dpw = persist.tile([128, EPC, CT, T], BF16)
            xd = persist.tile([128, EPC, HT, C], BF16)
            with (
                tc.tile_pool(name="pd", space="PSUM", bufs=2) as pd,
                tc.tile_pool(name="dsb", bufs=5) as dsb,
            ):
                for e in range(EPC):
                    Dts = []
                    for tt in range(TT):
                        Dt = dsb.tile([128, C], BF16, tag=f"D{tt}")
                        # (iota == pos) * mask
                        nc.vector.tensor_scalar(
                            Dt, iota_sb[:, :C],
                            pos[:, tt, e:e + 1], mask_f[:, tt, e:e + 1],
                            op0=ALU.is_equal, op1=ALU.mult,
                        )
                        Dts.append(Dt)
                        Dwt = dsb.tile([128, C], BF16, tag="Dw")
                        nc.vector.tensor_scalar_mul(
                            Dwt, Dt, cw[:, tt, e:e + 1]
                        )
                        for ct in range(CT):
                            tp = pd.tile([csz[ct], 128], BF16, tag="tp")
                            nc.tensor.transpose(
                                tp, Dwt[:, ds(ct * 128, csz[ct])], ident_bf
                            )
                            nc.vector.tensor_copy(
                                Dpw[:csz[ct], e, ct, ts(tt, 128)], tp
                            )
                    for hc in range(HT):
                        xd_ps = pd.tile([128, C], F32, tag="xd")
                        for tc_ in range(TT):
                            nc.tensor.matmul(
                                xd_ps,
                                x_sb[:, tc_, ts(hc, 128)],
                                Dts[tc_],
                                start=(tc_ == 0), stop=(tc_ == TT - 1),
                            )
                        nc.vector.tensor_copy(xd[:, e, hc], xd_ps)

            # ------------- experts: gate_up -> silu*up -> down -------------
            y_tiles = {}
            for e in range(EPC):
                h_sb = hpool.tile([128, IT, C], BF16, tag="h")
                with tc.tile_pool(name=f"pgu{e}", space="PSUM", bufs=4) as pgu:
                    for j in range(IT):
                        wg = wpool.tile([128, 2, H], BF16, tag="wgu")
                        nc.sync.dma_start(
                            wg, wgu_d[e, 2 * j:2 * j + 2].rearrange("a p f -> p a f")
                        )
                        ps_g = pgu.tile([128, C], F32, tag="gu")
                        ps_u = pgu.tile([128, C], F32, tag="gu")
                        for hc in range(HT):
                            nc.tensor.matmul(
                                ps_g, wg[:, 0, ts(hc, 128)], xd[:, e, hc],
                                start=(hc == 0), stop=(hc == HT - 1),
                            )
                        for hc in range(HT):
                            nc.tensor.matmul(
                                ps_u, wg[:, 1, ts(hc, 128)], xd[:, e, hc],
                                start=(hc == 0), stop=(hc == HT - 1),
                            )
                        sg = rsb.tile([128, C], BF16, tag="sg")
                        nc.scalar.activation(sg, ps_g, ACTF.Sigmoid)
                        sg2 = rsb.tile([128, C], BF16, tag="sg2")
                        nc.vector.tensor_mul(sg2, sg, ps_g)
                        nc.vector.tensor_mul(h_sb[:, j], sg2, ps_u)

                for ct in range(CT):
                    y_tiles[(e, ct)] = ypool.tile(
                        [csz[ct], H], BF16, tag=f"y{e}{ct}", name=f"y{e}{ct}"
                    )
                with tc.tile_pool(name=f"py{e}", space="PSUM", bufs=1) as py:
                    ps_y = {
                        (ct, hk): py.tile(
                            [csz[ct], 512], F32,
                            tag=f"py{ct}{hk}", name=f"psy{ct}{hk}",
                        )
                        for ct in range(CT) for hk in range(HK)
                    }
                    for ic in range(IT):
                        wdt = wpool.tile([128, H], BF16, tag="wd")
                        nc.sync.dma_start(wdt, wd_d[e, ic])
                        for ct in range(CT):
                            for hk in range(HK):
                                nc.tensor.matmul(
                                    ps_y[(ct, hk)],
                                    h_sb[:, ic, ds(ct * 128, csz[ct])],
                                    wdt[:, ts(hk, 512)],
                                    start=(ic == 0), stop=(ic == IT - 1),
                                )
                    for ct in range(CT):
                        for hk in range(HK):
                            nc.vector.tensor_copy(
                                y_tiles[(e, ct)][:, ts(hk, 512)], ps_y[(ct, hk)]
                            )

            # ------------- combine: routed (weighted) + shared -------------
            with (
                tc.tile_pool(name="po", space="PSUM", bufs=4) as po,
                tc.tile_pool(name="osb", bufs=4) as osb,
            ):
                chain = [("sh", j) for j in range(JSH)] + [
                    (e, ct) for e in range(EPC) for ct in range(CT)
                ]
                for tt in range(TT):
                    for hk in range(HK):
                        ps_o = po.tile([128, 512], F32, tag="o")
                        for n, (a, b) in enumerate(chain):
                            st, sp = (n == 0), (n == len(chain) - 1)
                            if a == "sh":
                                nc.tensor.matmul(
                                    ps_o,
                                    hsh[:, b, ts(tt, 128)],
                                    swd_sb[:, b, ts(hk, 512)],
                                    start=st, stop=sp,
                                )
                            else:
                                nc.tensor.matmul(
                                    ps_o,
                                    Dpw[:csz[b], a, b, ts(tt, 128)],
                                    y_tiles[(a, b)][:, ts(hk, 512)],
                                    start=st, stop=sp,
                                )
                        o_sb = osb.tile([128, 512], F32, tag="o")
                        nc.vector.tensor_copy(o_sb, ps_o)
                        nc.sync.dma_start(
                            out_d[ts(tt, 128), ts(hk, 512)], o_sb
                        )
    nc.compile()
    return nc


def _lhsT_tiles(Wt: np.ndarray, col0: int) -> np.ndarray:
    """Wt: [H, cols] fp32/bf16. Returns [128, H] where element (p, k*128+c) =
    Wt[k*128+p, col0+c] — i.e. the lhsT chunk layout for 16 h-chunks."""
    blk = Wt[:, col0:col0 + 128].reshape(HT, 128, 128)
    return np.ascontiguousarray(blk.transpose(1, 0, 2)).reshape(128, H)


def _route_capacity(x: np.ndarray, gate_w: np.ndarray) -> int:
    logits = x.astype(np.float64) @ gate_w.T.astype(np.float64)
    part = np.partition(logits, E - K - 1, axis=-1)
    thr = part[:, E - K - 1]  # (K+1)-th largest == just below the top-K
    counts = (logits > thr[:, None]).sum(0)
    c = int(counts.max()) + 8  # safety margin for fp32-vs-fp64 boundary flips
    return min(512, max(64, ((c + 31) // 32) * 32))


_BUILD_CACHE = {}


def prepare(
    hidden_states, gate_w, w_gate_up, w_down, shared_gate_up, shared_down
):
    """Host-side sharding/layout prep. Returns (C, in_maps)."""
    x = np.ascontiguousarray(np.asarray(hidden_states, dtype=np.float32))
    gate_w = np.asarray(gate_w, dtype=np.float32)
    w_gate_up = np.asarray(w_gate_up, dtype=np.float32)
    w_down = np.asarray(w_down, dtype=np.float32)
    shared_gate_up = np.asarray(shared_gate_up, dtype=np.float32)
    shared_down = np.asarray(shared_down, dtype=np.float32)

    C = _route_capacity(x, gate_w)

    # --- common (replicated) host-side layouts ---
    xt = np.ascontiguousarray(x.T)                        # [H, T]
    xt_f32 = xt.reshape(HT, 128, T)
    xt_bf = xt_f32.astype(BF)
    x_bf = x.reshape(TT, 128, H).astype(BF)
    tri = np.triu(np.ones((T, T), np.float32), 1).reshape(TT, 128, T).astype(BF)
    iota_row = np.broadcast_to(
        np.arange(T, dtype=np.float32), (128, T)
    ).copy()

    # shared expert: pad IS -> ISP and shard
    sg_T = np.zeros((H, ISP), np.float32)
    sg_T[:, :IS] = shared_gate_up[:IS].T
    su_T = np.zeros((H, ISP), np.float32)
    su_T[:, :IS] = shared_gate_up[IS:].T
    sd_T = np.zeros((ISP, H), np.float32)
    sd_T[:IS] = shared_down.T

    in_maps = []
    for c in range(NCORES):
        e0 = EPC * c
        # The device graph reads router columns 0..EPC-1 as "this core's
        # experts": permute gate_w rows so global experts (2c, 2c+1) land
        # in columns 0,1 (softmax/top-k/cumsum are column-order invariant).
        perm = [e0 + el for el in range(EPC)] + [
            e for e in range(E) if not (e0 <= e < e0 + EPC)
        ]
        gw_t = np.ascontiguousarray(gate_w[perm].T).reshape(HT, 128, E)
        wgu = np.empty((EPC, 2 * IT, 128, H), BF)
        wd = np.empty((EPC, IT, 128, H), BF)
        for el in range(EPC):
            Wt = w_gate_up[e0 + el].T.astype(np.float32)  # [H, 2I]
            for j in range(IT):
                wgu[el, 2 * j] = _lhsT_tiles(Wt, j * 128).astype(BF)
                wgu[el, 2 * j + 1] = _lhsT_tiles(Wt, I + j * 128).astype(BF)
            WdT = w_down[e0 + el].T  # [I, H]
            for ic in range(IT):
                wd[el, ic] = WdT[ic * 128:(ic + 1) * 128].astype(BF)
        base = c * ISC
        swgu = np.empty((2 * JSH, 128, H), BF)
        for j in range(JSH):
            swgu[2 * j] = _lhsT_tiles(sg_T, base + j * 128).astype(BF)
            swgu[2 * j + 1] = _lhsT_tiles(su_T, base + j * 128).astype(BF)
        swd = sd_T[base:base + ISC].reshape(JSH, 128, H).astype(BF)

        in_maps.append({
            "xt_f32": xt_f32, "gw_t": gw_t, "x_bf": x_bf, "xt_bf": xt_bf,
            "tri": tri, "iota_row": iota_row,
            "wgu": wgu, "wd": wd, "swgu": swgu, "swd": swd,
        })
    return C, in_maps


def kernel(**inputs):
    C, in_maps = prepare(**inputs)
    if C not in _BUILD_CACHE:
        _BUILD_CACHE[C] = build_nc(C)
    nc = _BUILD_CACHE[C]

    res = run_bass_kernel_spmd(nc, in_maps, core_ids=list(range(NCORES)))
    out = np.zeros((T, H), np.float32)
    for r in res.results:
        out += r["out"]
    return out
